# revision 22
# baseline (speedup 1.0000x reference)
"""BitStackLinear Trainium2 kernel (v2: dense-bf16 PE stream).

y = x @ w.T with w = sum_b sign_b * (u_b @ vt_b), signs bit-packed in qweight.

Column-parallel over out_features across 8 NeuronCores. Per 512-wide
out-feature group (og):

  Phase A (build w[:, og] bf16, one 128-row i'-chunk at a time):
    - PE: lr_b = vt_b.T-slice @ (-u_b) via 4 concurrent K=16 row-group
      matmuls (tile_position packing) -> -lr in PSUM
    - ACT: two half evictions PSUM->SBUF bf16 (pipelines the PSUM reuse)
    - DVE: mask = (q & (1<<j)) * 2^(15-j)  (sign bit at position 15),
      m_b = (-lr_b) XOR mask_b  (exact sign application, u16 bitwise)
    - Pool: t01 = m0+m1, t23 = m2+m3;  DVE: w = t01+t23

  Phase B (y[:, og] += x-chunk @ w-chunk): 4 token-slab accumulators in 4
  PSUM banks, 32 chunks per og, bf16 matmuls with 512-wide moving operand.

PE-density scheduling: phase A of og+1 is issue-interleaved into phase B
of og (1 A-step per few B-steps), so the cross-engine A pipeline runs in
the shadow of B's dense matmul stream and the PE never waits on w. For
og 0, B(tq=0) trickles directly behind A(0).

The contraction index is permuted bit-plane-major (i' = j*(I/8) + m) so
each 128-partition i'-chunk uses a single constant bit position j; x and
vt are permuted on the host to match.
"""
import sys

for _p in ("/opt/trn_rl_repo", "/root/.axon_site/_ro/trn_rl_repo"):
    if _p not in sys.path:
        sys.path.insert(0, _p)

import numpy as np
import ml_dtypes

import concourse.bass as bass
import concourse.tile as tile
from concourse import mybir
from concourse.bass_utils import run_bass_kernel_spmd

N_CORES = 8
B = 4       # bit planes
K = 16      # low-rank
T = 2048    # tokens
I = 4096    # in_features
O = 11008   # out_features
O_S = O // N_CORES  # 1376 per core

_SLOT = 512  # psum bank free width (f32)


def _og_chunks(o_s):
    out, o0 = [], 0
    while o0 < o_s:
        w = min(_SLOT, o_s - o0)
        out.append((o0, w))
        o0 += w
    return out


def build_nc(t=T, i=I, o_s=O_S):
    """Build the per-core SPMD Bass program (identical on all cores)."""
    nb = i // 8          # packed words per (b, o)
    mb_n = nb // 128     # byte-row blocks per bit plane
    nc_i = i // 128      # i'-chunks
    assert nc_i == 8 * mb_n and t % 512 == 0
    n_tq = t // 512      # token quads (512 tokens each, 4 psum slabs)
    ogs = _og_chunks(o_s)
    n_ogs = len(ogs)

    # phase A production order = phase B consumption order
    c_order = [j * mb_n + mb for mb in range(mb_n) for j in range(8)]

    nc = bass.Bass("TRN2", target_bir_lowering=False, debug=False)

    # x pre-tiled on host: [chunk, tq, 128, 512] so each (chunk, tq) tile is
    # one fully-contiguous 128KB DRAM block (the per-row 1KB strided loads
    # from a flat [i, t] layout starve the PE at full matmul rate)
    xt_d = nc.dram_tensor("xt", [nc_i, t // 512, 128, 512],
                          mybir.dt.bfloat16, kind="ExternalInput")
    # og-major packed q: per (mb, partition) the columns are
    # [og0: b0 ow0 | b1 ow0 | ... | og1: ...] so the per-og mask op reads a
    # flat contiguous 2D AP (DVE 4x mode needs step-1 innermost).
    qt_d = nc.dram_tensor("qt", [mb_n, 128, B * o_s], mybir.dt.uint16,
                          kind="ExternalInput")
    vt_d = nc.dram_tensor("vtp", [128, i], mybir.dt.bfloat16, kind="ExternalInput")
    ut_d = nc.dram_tensor("utp", [128, o_s], mybir.dt.bfloat16, kind="ExternalInput")
    y_d = nc.dram_tensor("y", [t, o_s], mybir.dt.float32, kind="ExternalOutput")

    f32 = mybir.dt.float32
    bf16 = mybir.dt.bfloat16
    u16 = mybir.dt.uint16
    AND = mybir.AluOpType.bitwise_and
    SHL = mybir.AluOpType.logical_shift_left
    XOR = mybir.AluOpType.bitwise_xor

    with tile.TileContext(nc) as tc:
        with (
            tc.tile_pool(name="const", bufs=1) as cpool,
            tc.tile_pool(name="w", bufs=1) as wpool,
            tc.tile_pool(name="q", bufs=2) as qpool,
            tc.tile_pool(name="mask", bufs=3) as maskpool,
            tc.tile_pool(name="lrsb", bufs=4) as lrsbpool,
            tc.tile_pool(name="m", bufs=3) as mpool,
            tc.tile_pool(name="tmp", bufs=2) as tmppool,
            tc.tile_pool(name="x", bufs=24) as xpool,
            tc.tile_pool(name="ysb", bufs=6) as ysbpool,
            tc.tile_pool(name="lrps", bufs=2, space="PSUM") as lrps,
            tc.tile_pool(name="yps", bufs=1, space="PSUM") as yps,
        ):
            # ---- persistent loads ----
            vt_sb = cpool.tile([128, i], bf16, tag="vt")
            nc.sync.dma_start(vt_sb[:], vt_d.ap())
            ut_sb = cpool.tile([128, o_s], bf16, tag="ut")
            nc.sync.dma_start(ut_sb[:], ut_d.ap())
            q_sb = []
            for mb in range(mb_n):
                q_t = qpool.tile([128, B * o_s], u16, name=f"q{mb}",
                                 tag=f"q{mb % 2}")
                nc.sync.dma_start(q_t[:], qt_d.ap()[mb])
                q_sb.append(q_t)
            # og-major column base offsets into q tiles
            og_qbase = []
            acc = 0
            for (_o0, _ow) in ogs:
                og_qbase.append(acc)
                acc += B * _ow

            w_tiles = [None] * n_ogs

            def a_steps(ogi):
                """Generator: one yield per i'-chunk of phase A for og ogi."""
                o0, ow = ogs[ogi]
                w_og = wpool.tile([128, nc_i * _SLOT], bf16, name=f"w{ogi}",
                                  tag=f"w{ogi % 2}")
                w_tiles[ogi] = w_og
                qb = og_qbase[ogi]
                for ci, c in enumerate(c_order):
                    j, mb = c // mb_n, c % mb_n
                    q_t = q_sb[mb]
                    # -- sub 0: mask + first plane-pair matmuls --
                    # sign-bit masks: ((q & (1<<j)) << (15-j)) -> {0, 0x8000};
                    # flat contiguous 2D AP (og-major q layout) for DVE 4x
                    mask_t = maskpool.tile([128, B * _SLOT], u16, name="mask_t",
                                           tag="mask")
                    nc.vector.tensor_scalar(
                        mask_t[:, : B * ow], q_t[:, qb:qb + B * ow],
                        1 << j, 15 - j, AND, SHL,
                    )
                    m_t = mpool.tile([128, B * _SLOT], u16, tag="m")
                    lr_h = [None, None]

                    def _mm_pair(h):
                        # -lr via K=16 row-group matmuls (ut = -u); two
                        # plane-pair PSUM tiles (2 banks each, bufs=2)
                        lr_ps = lrps.tile([128, 2 * _SLOT], f32, name="lr_ps",
                                          tag="lr_ps")
                        lr_h[h] = lr_ps
                        for bb in range(2):
                            b = 2 * h + bb
                            nc.tensor.matmul(
                                lr_ps[:, bb * _SLOT:bb * _SLOT + ow],
                                vt_sb[32 * b:32 * b + K, c * 128:(c + 1) * 128],
                                ut_sb[32 * b:32 * b + K, o0:o0 + ow],
                                start=True, stop=True,
                                tile_position=(32 * b, 0),
                            )

                    def _evict_xor(h):
                        lr_ps = lr_h[h]
                        lr_sb = lrsbpool.tile([128, 2 * _SLOT], bf16,
                                              tag="lrsb")
                        msl = slice(2 * h * _SLOT, 2 * (h + 1) * _SLOT)
                        if ow == _SLOT:
                            nc.scalar.copy(lr_sb[:], lr_ps[:])
                            nc.vector.tensor_tensor(
                                m_t[:, msl], lr_sb[:].bitcast(u16),
                                mask_t[:, msl], XOR,
                            )
                        else:
                            r = lambda ap: ap.rearrange(
                                "p (b w) -> p b w", b=2)[:, :, :ow]
                            nc.scalar.copy(r(lr_sb[:]), r(lr_ps[:]))
                            nc.vector.tensor_tensor(
                                r(m_t[:, msl]),
                                r(lr_sb[:]).bitcast(u16),
                                mask_t[:, 2 * h * ow:2 * (h + 1) * ow]
                                .rearrange("p (b w) -> p b w", b=2),
                                XOR,
                            )

                    # bunched issue (one c-step per yield): spreading these
                    # across B-slots was measured WORSE (all-engine ~20%
                    # per-instr inflation from sustained cross-engine overlap)
                    _mm_pair(0)
                    _evict_xor(0)
                    _mm_pair(1)
                    _evict_xor(1)
                    t01 = tmppool.tile([128, _SLOT], bf16, tag="t01")
                    t23 = tmppool.tile([128, _SLOT], bf16, tag="t23")
                    mb16 = m_t[:].bitcast(bf16)
                    nc.vector.tensor_add(
                        t01[:, :ow], mb16[:, 0:ow],
                        mb16[:, _SLOT:_SLOT + ow])
                    nc.gpsimd.tensor_add(
                        t23[:, :ow], mb16[:, 2 * _SLOT:2 * _SLOT + ow],
                        mb16[:, 3 * _SLOT:3 * _SLOT + ow])
                    nc.vector.tensor_add(
                        w_og[:, ci * _SLOT:ci * _SLOT + ow],
                        t01[:, :ow], t23[:, :ow],
                    )
                    yield

            ysums = [None] * 4

            def b_mms(ogi, tq, ci):
                """One phase-B step: x-chunk DMA + 4 token-slab matmuls."""
                o0, ow = ogs[ogi]
                c = c_order[ci]
                w_og = w_tiles[ogi]
                xt_t = xpool.tile([128, 512], bf16, tag="x")
                # two half-tile DMAs land on different queues (parallelism)
                nc.sync.dma_start(xt_t[0:64, :], xt_d.ap()[c, tq, 0:64, :])
                nc.sync.dma_start(xt_t[64:128, :], xt_d.ap()[c, tq, 64:128, :])
                for u in range(4):
                    nc.tensor.matmul(
                        ysums[u][:, :ow],
                        xt_t[:, u * 128:(u + 1) * 128],
                        w_og[:, ci * _SLOT:ci * _SLOT + ow],
                        start=(ci == 0), stop=(ci == nc_i - 1),
                    )

            def b_tq_open(tq):
                for u in range(4):
                    ysums[u] = yps.tile([128, _SLOT], f32, name=f"ysum{u}",
                                        tag=f"y{u}")

            def b_tq_close(ogi, tq):
                o0, ow = ogs[ogi]
                for u in range(4):
                    y_sb = ysbpool.tile([128, _SLOT], f32, tag="ysb")
                    nc.scalar.copy(y_sb[:, :ow], ysums[u][:, :ow])
                    nc.sync.dma_start(
                        y_d.ap()[(tq * 4 + u) * 128:(tq * 4 + u + 1) * 128,
                                 o0:o0 + ow],
                        y_sb[:, :ow],
                    )

            # ---- og 0 phase A, with B(0, tq=0) trickling right behind ----
            g0 = a_steps(0)
            b_tq_open(0)
            for ci in range(nc_i):
                next(g0)
                b_mms(0, 0, ci)
            b_tq_close(0, 0)

            # ---- steady state: B(og) with A(og+1) steps interleaved ----
            for ogi in range(n_ogs):
                tqs = list(range(1, n_tq)) if ogi == 0 else list(range(n_tq))
                nxt = a_steps(ogi + 1) if ogi + 1 < n_ogs else None
                n_slots = len(tqs) * nc_i
                acc = 0.0
                for tq in tqs:
                    b_tq_open(tq)
                    for ci in range(nc_i):
                        b_mms(ogi, tq, ci)
                        if nxt is not None:
                            acc += nc_i / n_slots
                            while acc >= 1.0:
                                next(nxt, None)
                                acc -= 1.0
                    b_tq_close(ogi, tq)
                if nxt is not None:
                    for _ in nxt:
                        pass

    _split_waits(nc)
    return nc


def _split_waits(nc, maxw=1):
    """This walrus build rejects instructions with more than a couple of
    sync-wait commands; move excess waits onto preceding same-engine NoOps."""
    for bb in nc.m.functions[0].blocks:
        insts = bb.instructions
        idx = 0
        while idx < len(insts):
            ins = insts[idx]
            si = ins.sync_info
            if si is not None and len(si.on_wait) > maxw:
                waits = list(si.on_wait)
                extra, keep = waits[:-maxw], waits[-maxw:]
                nops = []
                for k, wt in enumerate(extra):
                    nops.append(mybir.InstNoOp(
                        name=f"{ins.name}-wsplit{k}",
                        engine=ins.engine,
                        bass_nofuse=True,
                        sync_info=mybir.SyncInfo(on_wait=[wt], on_update=[]),
                    ))
                ins.sync_info = mybir.SyncInfo(on_wait=keep,
                                               on_update=list(si.on_update))
                for k, nop in enumerate(nops):
                    nc.register_instruction(nop, overwrite=True)
                    insts.insert(idx + k, nop)
                idx += len(nops)
            idx += 1


def prep_inputs(x, qweight, u, vt, n_cores=N_CORES):
    """Host-side layout prep + sharding. Returns (in_maps, meta)."""
    t, i = x.shape
    b_, o, k_ = u.shape
    nb = i // 8
    o_s = o // n_cores

    # x -> xt[i', t] bf16 with i' = j*(i/8) + m  (j-major bit-plane order),
    # then pre-tiled [chunk, tq, 128, 512] for contiguous 128KB tile DMAs
    xt = x.T.reshape(nb, 8, t).transpose(1, 0, 2).reshape(i, t)
    xt = np.ascontiguousarray(
        xt.reshape(i // 128, 128, t // 512, 512).transpose(0, 2, 1, 3)
    ).astype(ml_dtypes.bfloat16)

    # qweight -> qt[b, m, o] uint16 (byte-transposed; u16 ops hit the DVE
    # 16-bit packed mode, 2x the u8 rate)
    qt = np.ascontiguousarray(
        qweight.astype(np.uint16).reshape(b_, o, nb).transpose(0, 2, 1)
    )
    mb_n = nb // 128
    ogs = _og_chunks(o_s)

    # vt -> permuted + stacked into PE row groups [128, i], bf16
    vtp = vt.reshape(b_, k_, nb, 8).transpose(0, 1, 3, 2).reshape(b_, k_, i)
    vt_stack = np.zeros((128, i), np.float32)
    for b in range(b_):
        vt_stack[32 * b:32 * b + k_, :] = vtp[b]
    vt_stack = vt_stack.astype(ml_dtypes.bfloat16)

    # u -> -u^T stacked [128, o], bf16 (negated: sign applied by XOR with
    # bit mask, bit=1 flips -lr to +lr)
    ut_full = np.zeros((128, o), np.float32)
    for b in range(b_):
        ut_full[32 * b:32 * b + k_, :] = -u[b].T
    ut_full = ut_full.astype(ml_dtypes.bfloat16)

    in_maps = []
    for core in range(n_cores):
        o0 = core * o_s
        qc = qt[:, :, o0:o0 + o_s]            # [B, nb, o_s]
        # -> og-major packed [mb, 128, B*o_s]: per og slice, planes packed
        qparts = []
        for (go0, gow) in ogs:
            # [B, mb, 128, gow] -> [mb, 128, B, gow]
            sl = qc[:, :, go0:go0 + gow].reshape(b_, mb_n, 128, gow)
            qparts.append(sl.transpose(1, 2, 0, 3).reshape(mb_n, 128, b_ * gow))
        q_og = np.ascontiguousarray(np.concatenate(qparts, axis=2))
        in_maps.append({
            "xt": xt,
            "qt": q_og,
            "vtp": vt_stack,
            "utp": np.ascontiguousarray(ut_full[:, o0:o0 + o_s]),
        })
    return in_maps, (t, i, o, o_s)


_NC_CACHE = {}


def _get_nc(t, i, o_s):
    key = (t, i, o_s)
    if key not in _NC_CACHE:
        _NC_CACHE[key] = build_nc(t, i, o_s)
    return _NC_CACHE[key]


def run(x, qweight, u, vt, trace=False, **spmd_kwargs):
    in_maps, (t, i, o, o_s) = prep_inputs(x, qweight, u, vt)
    nc = _get_nc(t, i, o_s)
    res = run_bass_kernel_spmd(
        nc, in_maps, list(range(N_CORES)), trace=trace, **spmd_kwargs
    )
    y = np.concatenate([res.results[c]["y"] for c in range(N_CORES)], axis=1)
    return y, res


def kernel(x, qweight, u, vt):
    x = np.asarray(x, dtype=np.float32)
    qweight = np.asarray(qweight)
    u = np.asarray(u, dtype=np.float32)
    vt = np.asarray(vt, dtype=np.float32)
    y, _ = run(x, qweight, u, vt, trace=False)
    return y


# revision 23
# speedup vs baseline: 1.3095x; 1.3095x over previous
"""BitStackLinear Trainium2 kernel (v2: dense-bf16 PE stream).

y = x @ w.T with w = sum_b sign_b * (u_b @ vt_b), signs bit-packed in qweight.

Column-parallel over out_features across 8 NeuronCores. Per 512-wide
out-feature group (og):

  Phase A (build w[:, og] bf16, one 128-row i'-chunk at a time):
    - PE: lr_b = vt_b.T-slice @ (-u_b) via 4 concurrent K=16 row-group
      matmuls (tile_position packing) -> -lr in PSUM
    - ACT: two half evictions PSUM->SBUF bf16 (pipelines the PSUM reuse)
    - DVE: mask = (q & (1<<j)) * 2^(15-j)  (sign bit at position 15),
      m_b = (-lr_b) XOR mask_b  (exact sign application, u16 bitwise)
    - Pool: t01 = m0+m1, t23 = m2+m3;  DVE: w = t01+t23

  Phase B (y[:, og] += x-chunk @ w-chunk): 4 token-slab accumulators in 4
  PSUM banks, 32 chunks per og, bf16 matmuls with 512-wide moving operand.

PE-density scheduling: phase A of og+1 is issue-interleaved into phase B
of og (1 A-step per few B-steps), so the cross-engine A pipeline runs in
the shadow of B's dense matmul stream and the PE never waits on w. For
og 0, B(tq=0) trickles directly behind A(0).

The contraction index is permuted bit-plane-major (i' = j*(I/8) + m) so
each 128-partition i'-chunk uses a single constant bit position j; x and
vt are permuted on the host to match.
"""
import sys

for _p in ("/opt/trn_rl_repo", "/root/.axon_site/_ro/trn_rl_repo"):
    if _p not in sys.path:
        sys.path.insert(0, _p)

import numpy as np
import ml_dtypes

import concourse.bass as bass
import concourse.tile as tile
from concourse import mybir
from concourse.bass_utils import run_bass_kernel_spmd

N_CORES = 8
B = 4       # bit planes
K = 16      # low-rank
T = 2048    # tokens
I = 4096    # in_features
O = 11008   # out_features
O_S = O // N_CORES  # 1376 per core

_SLOT = 512  # psum bank free width (f32)


def _og_chunks(o_s):
    out, o0 = [], 0
    while o0 < o_s:
        w = min(_SLOT, o_s - o0)
        out.append((o0, w))
        o0 += w
    return out


def build_nc(t=T, i=I, o_s=O_S):
    """Build the per-core SPMD Bass program (identical on all cores)."""
    nb = i // 8          # packed words per (b, o)
    mb_n = nb // 128     # byte-row blocks per bit plane
    nc_i = i // 128      # i'-chunks
    assert nc_i == 8 * mb_n and t % 512 == 0
    n_tq = t // 512      # token quads (512 tokens each, 4 psum slabs)
    ogs = _og_chunks(o_s)
    n_ogs = len(ogs)

    # phase A production order = phase B consumption order
    c_order = [j * mb_n + mb for mb in range(mb_n) for j in range(8)]

    nc = bass.Bass("TRN2", target_bir_lowering=False, debug=False)

    # x pre-tiled on host: [chunk, tq, 128, 512] so each (chunk, tq) tile is
    # one fully-contiguous 128KB DRAM block (the per-row 1KB strided loads
    # from a flat [i, t] layout starve the PE at full matmul rate)
    xt_d = nc.dram_tensor("xt", [nc_i, t // 512, 128, 512],
                          mybir.dt.bfloat16, kind="ExternalInput")
    # og-major packed q: per (mb, partition) the columns are
    # [og0: b0 ow0 | b1 ow0 | ... | og1: ...] so the per-og mask op reads a
    # flat contiguous 2D AP (DVE 4x mode needs step-1 innermost).
    qt_d = nc.dram_tensor("qt", [mb_n, 128, B * o_s], mybir.dt.uint16,
                          kind="ExternalInput")
    vt_d = nc.dram_tensor("vtp", [128, i], mybir.dt.bfloat16, kind="ExternalInput")
    ut_d = nc.dram_tensor("utp", [128, o_s], mybir.dt.bfloat16, kind="ExternalInput")
    y_d = nc.dram_tensor("y", [t, o_s], mybir.dt.float32, kind="ExternalOutput")

    f32 = mybir.dt.float32
    bf16 = mybir.dt.bfloat16
    u16 = mybir.dt.uint16
    AND = mybir.AluOpType.bitwise_and
    SHL = mybir.AluOpType.logical_shift_left
    XOR = mybir.AluOpType.bitwise_xor

    with tile.TileContext(nc) as tc:
        with (
            tc.tile_pool(name="const", bufs=1) as cpool,
            tc.tile_pool(name="w", bufs=1) as wpool,
            tc.tile_pool(name="q", bufs=2) as qpool,
            tc.tile_pool(name="mask", bufs=3) as maskpool,
            tc.tile_pool(name="lrsb", bufs=4) as lrsbpool,
            tc.tile_pool(name="m", bufs=3) as mpool,
            tc.tile_pool(name="tmp", bufs=2) as tmppool,
            tc.tile_pool(name="x", bufs=24) as xpool,
            tc.tile_pool(name="ysb", bufs=6) as ysbpool,
            tc.tile_pool(name="lrps", bufs=2, space="PSUM") as lrps,
            tc.tile_pool(name="yps", bufs=1, space="PSUM") as yps,
        ):
            # ---- persistent loads ----
            vt_sb = cpool.tile([128, i], bf16, tag="vt")
            nc.sync.dma_start(vt_sb[:], vt_d.ap())
            ut_sb = cpool.tile([128, o_s], bf16, tag="ut")
            nc.sync.dma_start(ut_sb[:], ut_d.ap())
            q_sb = []
            for mb in range(mb_n):
                q_t = qpool.tile([128, B * o_s], u16, name=f"q{mb}",
                                 tag=f"q{mb % 2}")
                nc.sync.dma_start(q_t[:], qt_d.ap()[mb])
                q_sb.append(q_t)
            # og-major column base offsets into q tiles
            og_qbase = []
            acc = 0
            for (_o0, _ow) in ogs:
                og_qbase.append(acc)
                acc += B * _ow

            w_tiles = [None] * n_ogs

            def a_steps(ogi):
                """Generator: one yield per i'-chunk of phase A for og ogi."""
                o0, ow = ogs[ogi]
                w_og = wpool.tile([128, nc_i * _SLOT], bf16, name=f"w{ogi}",
                                  tag=f"w{ogi % 2}")
                w_tiles[ogi] = w_og
                qb = og_qbase[ogi]
                for ci, c in enumerate(c_order):
                    j, mb = c // mb_n, c % mb_n
                    q_t = q_sb[mb]
                    # -- sub 0: mask + first plane-pair matmuls --
                    # sign-bit masks: ((q & (1<<j)) << (15-j)) -> {0, 0x8000};
                    # flat contiguous 2D AP (og-major q layout) for DVE 4x
                    mask_t = maskpool.tile([128, B * _SLOT], u16, name="mask_t",
                                           tag="mask")
                    nc.vector.tensor_scalar(
                        mask_t[:, : B * ow], q_t[:, qb:qb + B * ow],
                        1 << j, 15 - j, AND, SHL,
                    )
                    m_t = mpool.tile([128, B * _SLOT], u16, tag="m")
                    lr_h = [None, None]

                    def _mm_pair(h):
                        # -lr via K=16 row-group matmuls (ut = -u); two
                        # plane-pair PSUM tiles (2 banks each, bufs=2)
                        lr_ps = lrps.tile([128, 2 * _SLOT], f32, name="lr_ps",
                                          tag="lr_ps")
                        lr_h[h] = lr_ps
                        for bb in range(2):
                            b = 2 * h + bb
                            nc.tensor.matmul(
                                lr_ps[:, bb * _SLOT:bb * _SLOT + ow],
                                vt_sb[32 * b:32 * b + K, c * 128:(c + 1) * 128],
                                ut_sb[32 * b:32 * b + K, o0:o0 + ow],
                                start=True, stop=True,
                                tile_position=(32 * b, 0),
                            )

                    def _evict_xor(h):
                        lr_ps = lr_h[h]
                        lr_sb = lrsbpool.tile([128, 2 * _SLOT], bf16,
                                              tag="lrsb")
                        msl = slice(2 * h * _SLOT, 2 * (h + 1) * _SLOT)
                        if ow == _SLOT:
                            nc.scalar.copy(lr_sb[:], lr_ps[:])
                            nc.vector.tensor_tensor(
                                m_t[:, msl], lr_sb[:].bitcast(u16),
                                mask_t[:, msl], XOR,
                            )
                        else:
                            r = lambda ap: ap.rearrange(
                                "p (b w) -> p b w", b=2)[:, :, :ow]
                            nc.scalar.copy(r(lr_sb[:]), r(lr_ps[:]))
                            nc.vector.tensor_tensor(
                                r(m_t[:, msl]),
                                r(lr_sb[:]).bitcast(u16),
                                mask_t[:, 2 * h * ow:2 * (h + 1) * ow]
                                .rearrange("p (b w) -> p b w", b=2),
                                XOR,
                            )

                    # bunched issue (one c-step per yield): spreading these
                    # across B-slots was measured WORSE (all-engine ~20%
                    # per-instr inflation from sustained cross-engine overlap)
                    _mm_pair(0)
                    _evict_xor(0)
                    _mm_pair(1)
                    _evict_xor(1)
                    t01 = tmppool.tile([128, _SLOT], bf16, tag="t01")
                    t23 = tmppool.tile([128, _SLOT], bf16, tag="t23")
                    mb16 = m_t[:].bitcast(bf16)
                    nc.vector.tensor_add(
                        t01[:, :ow], mb16[:, 0:ow],
                        mb16[:, _SLOT:_SLOT + ow])
                    nc.gpsimd.tensor_add(
                        t23[:, :ow], mb16[:, 2 * _SLOT:2 * _SLOT + ow],
                        mb16[:, 3 * _SLOT:3 * _SLOT + ow])
                    nc.vector.tensor_add(
                        w_og[:, ci * _SLOT:ci * _SLOT + ow],
                        t01[:, :ow], t23[:, :ow],
                    )
                    yield

            ysums = [None] * 4

            def b_mms(ogi, tq, ci):
                """One phase-B step: x-chunk DMA + 4 token-slab matmuls."""
                o0, ow = ogs[ogi]
                c = c_order[ci]
                w_og = w_tiles[ogi]
                xt_t = xpool.tile([128, 512], bf16, tag="x")
                nc.sync.dma_start(xt_t[:], xt_d.ap()[c, tq])
                for u in range(4):
                    nc.tensor.matmul(
                        ysums[u][:, :ow],
                        xt_t[:, u * 128:(u + 1) * 128],
                        w_og[:, ci * _SLOT:ci * _SLOT + ow],
                        start=(ci == 0), stop=(ci == nc_i - 1),
                    )

            def b_tq_open(tq):
                for u in range(4):
                    ysums[u] = yps.tile([128, _SLOT], f32, name=f"ysum{u}",
                                        tag=f"y{u}")

            def b_tq_close(ogi, tq):
                o0, ow = ogs[ogi]
                for u in range(4):
                    y_sb = ysbpool.tile([128, _SLOT], f32, tag="ysb")
                    nc.scalar.copy(y_sb[:, :ow], ysums[u][:, :ow])
                    nc.sync.dma_start(
                        y_d.ap()[(tq * 4 + u) * 128:(tq * 4 + u + 1) * 128,
                                 o0:o0 + ow],
                        y_sb[:, :ow],
                    )

            # ---- og 0 phase A, with B(0, tq=0) trickling right behind ----
            g0 = a_steps(0)
            b_tq_open(0)
            for ci in range(nc_i):
                next(g0)
                b_mms(0, 0, ci)
            b_tq_close(0, 0)

            # ---- steady state: B(og) with A(og+1) steps interleaved ----
            for ogi in range(n_ogs):
                tqs = list(range(1, n_tq)) if ogi == 0 else list(range(n_tq))
                nxt = a_steps(ogi + 1) if ogi + 1 < n_ogs else None
                n_slots = len(tqs) * nc_i
                acc = 0.0
                for tq in tqs:
                    b_tq_open(tq)
                    for ci in range(nc_i):
                        b_mms(ogi, tq, ci)
                        if nxt is not None:
                            acc += nc_i / n_slots
                            while acc >= 1.0:
                                next(nxt, None)
                                acc -= 1.0
                    b_tq_close(ogi, tq)
                if nxt is not None:
                    for _ in nxt:
                        pass

    _split_waits(nc)
    return nc


def _split_waits(nc, maxw=1):
    """This walrus build rejects instructions with more than a couple of
    sync-wait commands; move excess waits onto preceding same-engine NoOps."""
    for bb in nc.m.functions[0].blocks:
        insts = bb.instructions
        idx = 0
        while idx < len(insts):
            ins = insts[idx]
            si = ins.sync_info
            if si is not None and len(si.on_wait) > maxw:
                waits = list(si.on_wait)
                extra, keep = waits[:-maxw], waits[-maxw:]
                nops = []
                for k, wt in enumerate(extra):
                    nops.append(mybir.InstNoOp(
                        name=f"{ins.name}-wsplit{k}",
                        engine=ins.engine,
                        bass_nofuse=True,
                        sync_info=mybir.SyncInfo(on_wait=[wt], on_update=[]),
                    ))
                ins.sync_info = mybir.SyncInfo(on_wait=keep,
                                               on_update=list(si.on_update))
                for k, nop in enumerate(nops):
                    nc.register_instruction(nop, overwrite=True)
                    insts.insert(idx + k, nop)
                idx += len(nops)
            idx += 1


def prep_inputs(x, qweight, u, vt, n_cores=N_CORES):
    """Host-side layout prep + sharding. Returns (in_maps, meta)."""
    t, i = x.shape
    b_, o, k_ = u.shape
    nb = i // 8
    o_s = o // n_cores

    # x -> xt[i', t] bf16 with i' = j*(i/8) + m  (j-major bit-plane order),
    # then pre-tiled [chunk, tq, 128, 512] for contiguous 128KB tile DMAs
    xt = x.T.reshape(nb, 8, t).transpose(1, 0, 2).reshape(i, t)
    xt = np.ascontiguousarray(
        xt.reshape(i // 128, 128, t // 512, 512).transpose(0, 2, 1, 3)
    ).astype(ml_dtypes.bfloat16)

    # qweight -> qt[b, m, o] uint16 (byte-transposed; u16 ops hit the DVE
    # 16-bit packed mode, 2x the u8 rate)
    qt = np.ascontiguousarray(
        qweight.astype(np.uint16).reshape(b_, o, nb).transpose(0, 2, 1)
    )
    mb_n = nb // 128
    ogs = _og_chunks(o_s)

    # vt -> permuted + stacked into PE row groups [128, i], bf16
    vtp = vt.reshape(b_, k_, nb, 8).transpose(0, 1, 3, 2).reshape(b_, k_, i)
    vt_stack = np.zeros((128, i), np.float32)
    for b in range(b_):
        vt_stack[32 * b:32 * b + k_, :] = vtp[b]
    vt_stack = vt_stack.astype(ml_dtypes.bfloat16)

    # u -> -u^T stacked [128, o], bf16 (negated: sign applied by XOR with
    # bit mask, bit=1 flips -lr to +lr)
    ut_full = np.zeros((128, o), np.float32)
    for b in range(b_):
        ut_full[32 * b:32 * b + k_, :] = -u[b].T
    ut_full = ut_full.astype(ml_dtypes.bfloat16)

    in_maps = []
    for core in range(n_cores):
        o0 = core * o_s
        qc = qt[:, :, o0:o0 + o_s]            # [B, nb, o_s]
        # -> og-major packed [mb, 128, B*o_s]: per og slice, planes packed
        qparts = []
        for (go0, gow) in ogs:
            # [B, mb, 128, gow] -> [mb, 128, B, gow]
            sl = qc[:, :, go0:go0 + gow].reshape(b_, mb_n, 128, gow)
            qparts.append(sl.transpose(1, 2, 0, 3).reshape(mb_n, 128, b_ * gow))
        q_og = np.ascontiguousarray(np.concatenate(qparts, axis=2))
        in_maps.append({
            "xt": xt,
            "qt": q_og,
            "vtp": vt_stack,
            "utp": np.ascontiguousarray(ut_full[:, o0:o0 + o_s]),
        })
    return in_maps, (t, i, o, o_s)


_NC_CACHE = {}


def _get_nc(t, i, o_s):
    key = (t, i, o_s)
    if key not in _NC_CACHE:
        _NC_CACHE[key] = build_nc(t, i, o_s)
    return _NC_CACHE[key]


def run(x, qweight, u, vt, trace=False, **spmd_kwargs):
    in_maps, (t, i, o, o_s) = prep_inputs(x, qweight, u, vt)
    nc = _get_nc(t, i, o_s)
    res = run_bass_kernel_spmd(
        nc, in_maps, list(range(N_CORES)), trace=trace, **spmd_kwargs
    )
    y = np.concatenate([res.results[c]["y"] for c in range(N_CORES)], axis=1)
    return y, res


def kernel(x, qweight, u, vt):
    x = np.asarray(x, dtype=np.float32)
    qweight = np.asarray(qweight)
    u = np.asarray(u, dtype=np.float32)
    vt = np.asarray(vt, dtype=np.float32)
    y, _ = run(x, qweight, u, vt, trace=False)
    return y
